# revision 34
# baseline (speedup 1.0000x reference)
"""Mamba block (MockMambaBlock) on 8 Trainium2 NeuronCores.

Sharding: tensor-parallel over d_inner (8 x 256 channels), both batches on
every core. The x_proj/dt_proj contraction over d_inner is completed with an
on-device AllReduce of the small (B, 32, L) partial; out_proj row-partials
are summed on the host (the gather step).

v4: conv on the PE (per-tap diagonal matmuls); AllReduce split into L/2
halves so the scan chain starts ~40us earlier; full-width (N=2048) identity
and diagd matmuls into a single 4-bank PSUM accumulator; first/last blocks
scanned in chained halves so gating + out_proj pipeline into head/tail;
phase A of batch 1 emitted interleaved with the batch-0 scan chain.
"""

import sys

sys.path.insert(0, "/opt/trn_rl_repo")

import numpy as np
import ml_dtypes

import concourse.bass as bass
import concourse.bacc as bacc
import concourse.mybir as mybir
import concourse.tile as tile
from concourse.bass_utils import run_bass_kernel_spmd

F32 = mybir.dt.float32
BF16 = mybir.dt.bfloat16
AF = mybir.ActivationFunctionType
OP = mybir.AluOpType

B, L, DM, DI, DS, DC = 2, 2048, 1024, 2048, 16, 4
NCORES = 8
DIL = DI // NCORES          # 256 channels per core
NBLK = DIL // 128           # 2 partition blocks of channels
KBLK = DM // 128            # 8 contraction blocks for in_proj
LTA = 512                   # phase A token chunk
LH = L // 2                 # AllReduce / scan half


def build_nc():
    nc = bacc.Bacc()

    x_t = nc.dram_tensor("x_t", [B, KBLK, 128, L], BF16, kind="ExternalInput")
    win_d = nc.dram_tensor("win", [DM, 2 * DIL], BF16, kind="ExternalInput")
    wout_d = nc.dram_tensor("wout", [DIL, DM], BF16, kind="ExternalInput")
    wx_d = nc.dram_tensor("wx", [DIL, 2 * DS], BF16, kind="ExternalInput")
    wdt_d = nc.dram_tensor("wdt", [DS, DIL], BF16, kind="ExternalInput")
    a_d = nc.dram_tensor("a", [DIL, DS], F32, kind="ExternalInput")
    convb_d = nc.dram_tensor("convb", [DIL, 1], F32, kind="ExternalInput")
    convw_d = nc.dram_tensor("convw", [DIL, DC], F32, kind="ExternalInput")
    bdt_d = nc.dram_tensor("bdt", [DIL, 1], F32, kind="ExternalInput")
    identb_d = nc.dram_tensor("identb", [128, 128], BF16, kind="ExternalInput")
    diagd_d = nc.dram_tensor("diagd", [DIL, 128], BF16, kind="ExternalInput")
    diagk_d = nc.dram_tensor("diagk", [DC, DIL, 128], BF16, kind="ExternalInput")
    out_d = nc.dram_tensor("out_p", [B, L, DM], F32, kind="ExternalOutput")

    ncha = L // LTA

    with tile.TileContext(nc) as tc:
        with (
            tc.tile_pool(name="weights", bufs=1) as wp,
            tc.tile_pool(name="resident", bufs=1) as rp,
            tc.tile_pool(name="dram", bufs=1, space="DRAM") as dp,
        ):
            # ---- weights to SBUF ----
            win_sb = wp.tile([128, KBLK, 2 * DIL], BF16)
            nc.sync.dma_start(win_sb[:], win_d[:].rearrange("(k p) m -> p k m", p=128))
            wout_sb = wp.tile([128, NBLK, DM], BF16)
            nc.sync.dma_start(wout_sb[:], wout_d[:].rearrange("(k p) m -> p k m", p=128))
            wx_sb = wp.tile([128, NBLK, 2 * DS], BF16)
            nc.sync.dma_start(wx_sb[:], wx_d[:].rearrange("(k p) m -> p k m", p=128))
            wdt_sb = wp.tile([DS, DIL], BF16)
            nc.sync.dma_start(wdt_sb[:], wdt_d[:])
            a_sb = wp.tile([128, NBLK, DS], F32)
            nc.sync.dma_start(a_sb[:], a_d[:].rearrange("(k p) m -> p k m", p=128))
            convb_sb = wp.tile([128, NBLK, 1], F32)
            nc.sync.dma_start(convb_sb[:], convb_d[:].rearrange("(k p) m -> p k m", p=128))
            convw_sb = wp.tile([128, NBLK, DC], F32)
            nc.sync.dma_start(convw_sb[:], convw_d[:].rearrange("(k p) m -> p k m", p=128))
            bdt_sb = wp.tile([128, NBLK, 1], F32)
            nc.sync.dma_start(bdt_sb[:], bdt_d[:].rearrange("(k p) m -> p k m", p=128))
            identb_sb = wp.tile([128, 128], BF16)
            nc.sync.dma_start(identb_sb[:], identb_d[:])
            diagd_sb = wp.tile([128, NBLK, 128], BF16)
            nc.sync.dma_start(diagd_sb[:], diagd_d[:].rearrange("(k p) m -> p k m", p=128))
            diagk_sb = wp.tile([128, DC, NBLK, 128], BF16)
            nc.sync.dma_start(
                diagk_sb[:],
                diagk_d[:].rearrange("c (k p) m -> p c k m", p=128))

            # ---- resident activations (both batches) ----
            xcv = [[rp.tile([128, L], BF16, name=f"xcv{b_}{k}", tag=f"xcv{b_}{k}")
                    for k in range(NBLK)] for b_ in range(B)]
            zac = [[rp.tile([128, L], BF16, name=f"zac{b_}{k}", tag=f"zac{b_}{k}")
                    for k in range(NBLK)] for b_ in range(B)]
            md = [[rp.tile([128, L], BF16, name=f"md{b_}{k}", tag=f"md{b_}{k}")
                   for k in range(NBLK)] for b_ in range(B)]
            dtin_sb = [rp.tile([DS, L], BF16, name=f"dtin{b_}", tag=f"dtin{b_}")
                       for b_ in range(B)]
            xp = [[rp.tile([128, L + DC - 1], BF16, name=f"xp{b_}{k}",
                           tag=f"xp{b_}{k}") for k in range(NBLK)]
                  for b_ in range(B)]
            yin = [[rp.tile([128, L], BF16, name=f"yin{b_}{k}", tag=f"yin{b_}{k}")
                    for k in range(NBLK)] for b_ in range(B)]

            # collective buffers, one per (batch, token-range). Batch 0 uses
            # finer leading ranges so its scan chain can start early.
            RNG = {0: [(0, 512), (512, 1024), (1024, 2048)],
                   1: [(0, 1024), (1024, 2048)]}
            cc_in = {b_: [dp.tile([2 * DS, r1 - r0], BF16,
                                  name=f"cc_in{b_}_{r0}")
                          for (r0, r1) in RNG[b_]] for b_ in range(B)}
            cc_out = {b_: [dp.tile([2 * DS, r1 - r0], BF16,
                                   addr_space="Shared", name=f"cc_out{b_}_{r0}")
                           for (r0, r1) in RNG[b_]] for b_ in range(B)}

            # PSUM budget (8 banks): ps_in(2) + cps(1) + ps_xs(1) + y_ps(4).
            # ps_dt and ps_o reuse the ps_in tag.
            pools_cm = (
                tc.tile_pool(name="pa", bufs=2),
                tc.tile_pool(name="pa_ps", bufs=2, space="PSUM"),
                tc.tile_pool(name="pb", bufs=2),
                tc.tile_pool(name="pb_ps", bufs=1, space="PSUM"),
            )
            pa = pools_cm[0].__enter__()
            paps = pools_cm[1].__enter__()
            pb = pools_cm[2].__enter__()
            pbps = pools_cm[3].__enter__()

            def phase_a_chunk(b_, ch):
                t0 = ch * LTA
                xs_all = pa.tile([128, KBLK, LTA], BF16, tag="xs_all", bufs=3)
                nc.sync.dma_start(
                    xs_all[:],
                    x_t[b_].transpose([1, 0, 2])[:, :, t0:t0 + LTA])
                for m in range(2 * NBLK):
                    ps = paps.tile([128, LTA], F32, tag="ps_in", bufs=2)
                    for kb in range(KBLK):
                        nc.tensor.matmul(
                            ps[:],
                            win_sb[:, kb, m * 128:(m + 1) * 128],
                            xs_all[:, kb, :],
                            start=(kb == 0), stop=(kb == KBLK - 1))
                    if m < NBLK:  # x branch: conv (PE diag matmuls, or DVE
                        # scalar_tensor_tensor for batch 0 where DVE idles)
                        blk = m
                        if ch == 0:
                            nc.vector.memset(xp[b_][blk][:, 0:DC - 1], 0.0)
                        nc.scalar.copy(
                            xp[b_][blk][:, DC - 1 + t0:DC - 1 + t0 + LTA], ps[:])
                        if b_ == 0:
                            cacc = pa.tile([128, LTA], F32, tag="cacc", bufs=2)
                            nc.vector.tensor_scalar_mul(
                                cacc[:], xp[b_][blk][:, t0:t0 + LTA],
                                convw_sb[:, blk, 0:1])
                            for k in range(1, DC):
                                nc.vector.scalar_tensor_tensor(
                                    cacc[:], xp[b_][blk][:, t0 + k:t0 + k + LTA],
                                    convw_sb[:, blk, k:k + 1], cacc[:],
                                    OP.mult, OP.add)
                            nc.scalar.activation(
                                xcv[b_][blk][:, t0:t0 + LTA], cacc[:],
                                AF.Silu, bias=convb_sb[:, blk, :])
                        else:
                            cps = paps.tile([128, LTA], F32, tag="cps", bufs=1)
                            for k in range(DC):
                                nc.tensor.matmul(
                                    cps[:],
                                    diagk_sb[:, k, blk, :],
                                    xp[b_][blk][:, t0 + k:t0 + k + LTA],
                                    start=(k == 0), stop=(k == DC - 1))
                            nc.scalar.activation(
                                xcv[b_][blk][:, t0:t0 + LTA], cps[:],
                                AF.Silu, bias=convb_sb[:, blk, :])
                    else:  # z branch: silu
                        blk = m - NBLK
                        nc.scalar.activation(
                            zac[b_][blk][:, t0:t0 + LTA], ps[:], AF.Silu)
                # x_proj partial for this chunk
                ps_xs = paps.tile([2 * DS, LTA], F32, tag="ps_xs", bufs=1)
                for kb in range(NBLK):
                    nc.tensor.matmul(
                        ps_xs[:],
                        wx_sb[:, kb, :],
                        xcv[b_][kb][:, t0:t0 + LTA],
                        start=(kb == 0), stop=(kb == NBLK - 1))
                xs_sb = pa.tile([2 * DS, LTA], BF16, tag="xs_sb", bufs=2)
                nc.scalar.copy(xs_sb[:], ps_xs[:])
                # scalar HWDGE queue: stays clear of the big xs_all loads
                for ri, (r0, r1) in enumerate(RNG[b_]):
                    if r0 <= t0 < r1:
                        nc.scalar.dma_start(
                            cc_in[b_][ri][:, t0 - r0:t0 - r0 + LTA], xs_sb[:])

            def all_reduce(b_, ri):
                r0, r1 = RNG[b_][ri]
                nc.gpsimd.collective_compute(
                    "AllReduce", OP.add,
                    ins=[cc_in[b_][ri].opt()], outs=[cc_out[b_][ri].opt()],
                    replica_groups=[list(range(NCORES))])
                nc.scalar.dma_start(dtin_sb[b_][:, r0:r1],
                                    cc_out[b_][ri][0:DS, :])

            def dt_phase(b_, ri):
                # md = -softplus(dt_raw + b_dt) = ln(sigmoid(-(dt_raw + b_dt)))
                LTD = 512
                r0, r1 = RNG[b_][ri]
                for blk in range(NBLK):
                    for ch in range((r1 - r0) // LTD):
                        t0 = r0 + ch * LTD
                        ps_dt = paps.tile([128, LTD], F32, tag="ps_in", bufs=2)
                        nc.tensor.matmul(
                            ps_dt[:], wdt_sb[:, blk * 128:(blk + 1) * 128],
                            dtin_sb[b_][:, t0:t0 + LTD],
                            start=True, stop=True)
                        nc.scalar.activation(
                            md[b_][blk][:, t0:t0 + LTD], ps_dt[:],
                            AF.Sigmoid, bias=bdt_sb[:, blk, :], scale=-1.0)
                for blk in range(NBLK):
                    nc.scalar.activation(md[b_][blk][:, r0:r1],
                                         md[b_][blk][:, r0:r1], AF.Ln)

            def make_dtx(b_, blk, on_dve=False, t0=0, t1=L, dtx=None):
                if dtx is None:
                    dtx = pb.tile([128, L], BF16, tag="dtx", bufs=2,
                                  name=f"dtx{b_}{blk}")
                eng = nc.vector if on_dve else nc.gpsimd
                eng.tensor_mul(dtx[:, t0:t1], md[b_][blk][:, t0:t1],
                               xcv[b_][blk][:, t0:t1])
                return dtx

            def make_yps(b_, blk):
                y_ps = pbps.tile([128, L], F32, tag="y_ps", bufs=1,
                                 name=f"yps{b_}{blk}")
                for pt in range(L // 512):
                    nc.tensor.matmul(y_ps[:, pt * 512:(pt + 1) * 512],
                                     diagd_sb[:, blk, :],
                                     xcv[b_][blk][:, pt * 512:(pt + 1) * 512],
                                     start=True, stop=False)
                return y_ps

            # states whose y-accumulation runs as DVE tree-adds instead of PE
            # identity matmuls (PE/DVE load balance)
            DVE_SUM = set(range(DS - 4, DS))

            def phase_b_n(b_, blk, n, dtx, y_ps, t0=0, t1=L, carry=None,
                          save_carry=None, hstate=None):
                tl = t1 - t0
                bb = pb.tile([128, tl], BF16, tag="bbn", bufs=3,
                             name=f"bb{b_}{blk}{n}{t0}")
                done = 0
                for ri, (r0, r1) in enumerate(RNG[b_]):
                    o0, o1 = max(t0, r0), min(t1, r1)
                    if o0 < o1:
                        # pool SWDGE queue: avoids contention with xs_all loads
                        nc.gpsimd.dma_start(
                            bb[:, o0 - t0:o1 - t0],
                            cc_out[b_][ri][DS + n:DS + n + 1, o0 - r0:o1 - r0]
                            .broadcast_to([128, o1 - o0]))
                        done += o1 - o0
                assert done == tl
                da = pb.tile([128, tl], F32, tag="dan", bufs=2,
                             name=f"da{b_}{blk}{n}{t0}")
                nc.scalar.activation(da[:], md[b_][blk][:, t0:t1], AF.Exp,
                                     scale=a_sb[:, blk, n:n + 1])
                u = pb.tile([128, tl], BF16, tag="un", bufs=3,
                            name=f"u{b_}{blk}{n}{t0}")
                nc.vector.tensor_mul(u[:], dtx[:, t0:t1], bb[:])
                h = pb.tile([128, tl], BF16, tag="hn", bufs=2,
                            name=f"h{b_}{blk}{n}{t0}")
                init = 0.0 if carry is None else carry[:, n:n + 1]
                nc.vector.tensor_tensor_scan(h[:], da[:], u[:],
                                             init, OP.mult, OP.add)
                if save_carry is not None:
                    nc.vector.tensor_copy(save_carry[:, n:n + 1], h[:, tl - 1:tl])
                if n in DVE_SUM:
                    # fold into the DVE partial sum; last state emits the
                    # shadow identity matmul with the stop flag
                    first = min(DVE_SUM)
                    if n == first:
                        hstate["h0"] = h
                    elif n == first + 1:
                        S = pb.tile([128, tl], BF16, tag="hsum", bufs=2,
                                    name=f"hs{b_}{blk}{t0}")
                        nc.vector.tensor_add(S[:], hstate.pop("h0")[:], h[:])
                        hstate["S"] = S
                    else:
                        S = hstate["S"]
                        nc.vector.tensor_add(S[:], S[:], h[:])
                    if n == DS - 1:
                        S = hstate.pop("S")
                        for pt in range(tl // 512):
                            nc.tensor.matmul(
                                y_ps[:, t0 + pt * 512:t0 + (pt + 1) * 512],
                                identb_sb[:], S[:, pt * 512:(pt + 1) * 512],
                                start=False, stop=True)
                else:
                    for pt in range(tl // 512):
                        nc.tensor.matmul(
                            y_ps[:, t0 + pt * 512:t0 + (pt + 1) * 512],
                            identb_sb[:], h[:, pt * 512:(pt + 1) * 512],
                            start=False, stop=False)

            def phase_b_gate(b_, blk, y_ps, t0=0, t1=L):
                nc.vector.tensor_mul(
                    yin[b_][blk][:, t0:t1], y_ps[:, t0:t1],
                    zac[b_][blk][:, t0:t1])

            def out_proj(b_, mts):
                for mt in mts:
                    for dmh in range(2):
                        ps_o = paps.tile([128, 512], F32, tag="ps_in", bufs=2)
                        for blk in range(NBLK):
                            nc.tensor.matmul(
                                ps_o[:],
                                yin[b_][blk][:, mt * 128:(mt + 1) * 128],
                                wout_sb[:, blk, dmh * 512:(dmh + 1) * 512],
                                start=(blk == 0), stop=(blk == NBLK - 1))
                        osb = pb.tile([128, 512], F32, tag="osb", bufs=2)
                        nc.scalar.copy(osb[:], ps_o[:])
                        nc.sync.dma_start(
                            out_d[b_, mt * 128:(mt + 1) * 128,
                                  dmh * 512:(dmh + 1) * 512],
                            osb[:])

            # ---------------- emission schedule ----------------
            phase_a_chunk(0, 0)
            all_reduce(0, 0)
            phase_a_chunk(0, 1)
            all_reduce(0, 1)
            phase_a_chunk(0, 2)
            phase_a_chunk(0, 3)
            all_reduce(0, 2)
            dt_phase(0, 0)
            dtx00 = make_dtx(0, 0, on_dve=True, t0=0, t1=512)
            y00 = make_yps(0, 0)
            carry00 = pb.tile([128, DS], BF16, tag="carry", bufs=2,
                              name="carry00")
            # B(0,0) segment 1 as early as possible
            hs = {}
            for n in range(DS):
                phase_b_n(0, 0, n, dtx00, y00, 0, 512, save_carry=carry00,
                          hstate=hs)
            dt_phase(0, 1)
            make_dtx(0, 0, on_dve=True, t0=512, t1=1024, dtx=dtx00)
            phase_b_gate(0, 0, y00, 0, 512)
            hs = {}
            for n in range(DS):
                phase_b_n(0, 0, n, dtx00, y00, 512, 1024, carry=carry00,
                          save_carry=carry00, hstate=hs)
            dt_phase(0, 2)
            make_dtx(0, 0, on_dve=True, t0=1024, t1=L, dtx=dtx00)
            phase_b_gate(0, 0, y00, 512, 1024)
            # segment 3 interleaved with phase A of batch 1
            hs = {}
            nxt = 0
            for ch in range(ncha):
                phase_a_chunk(1, ch)
                if ch == 1:
                    all_reduce(1, 0)
                if ch == 3:
                    all_reduce(1, 1)
                for n in range(nxt, nxt + 4):
                    phase_b_n(0, 0, n, dtx00, y00, 1024, L, carry=carry00,
                              hstate=hs)
                nxt += 4
            phase_b_gate(0, 0, y00, 1024, L)
            # B(0,1) full-length
            dtx01 = make_dtx(0, 1)
            y01 = make_yps(0, 1)
            hs = {}
            phase_b_n(0, 1, 0, dtx01, y01, hstate=hs)
            phase_b_n(0, 1, 1, dtx01, y01, hstate=hs)
            dt_phase(1, 0)
            dt_phase(1, 1)
            for n in range(2, DS):
                phase_b_n(0, 1, n, dtx01, y01, hstate=hs)
            phase_b_gate(0, 1, y01)
            # B(1,0) with out_proj(0) interleaved
            dtx10 = make_dtx(1, 0)
            y10 = make_yps(1, 0)
            hs = {}
            for n in range(DS):
                phase_b_n(1, 0, n, dtx10, y10, hstate=hs)
                if n < 8:
                    out_proj(0, mts=[2 * n, 2 * n + 1])
            phase_b_gate(1, 0, y10)
            # B(1,1) in chained halves; out_proj(1) rides the second half
            dtx11 = make_dtx(1, 1)
            y11 = make_yps(1, 1)
            carry11 = pb.tile([128, DS], BF16, tag="carry", bufs=2,
                              name="carry11")
            hs = {}
            for n in range(DS):
                phase_b_n(1, 1, n, dtx11, y11, 0, LH, save_carry=carry11,
                          hstate=hs)
            phase_b_gate(1, 1, y11, 0, LH)
            hs = {}
            for n in range(DS):
                phase_b_n(1, 1, n, dtx11, y11, LH, L, carry=carry11,
                          hstate=hs)
                if n < 8:
                    out_proj(1, mts=[2 * n, 2 * n + 1])
            phase_b_gate(1, 1, y11, LH, L)
            out_proj(1, mts=range(8, 16))

            for cm in reversed(pools_cm):
                cm.__exit__(None, None, None)

    nc.compile()
    return nc


_NC_CACHE = {}


def _get_nc():
    if "nc" not in _NC_CACHE:
        _NC_CACHE["nc"] = build_nc()
    return _NC_CACHE["nc"]


def make_in_maps(x, W_in, conv_w, conv_b, W_x, W_dt, b_dt, A_log, D, W_out):
    x = np.asarray(x, np.float32)
    W_in = np.asarray(W_in, np.float32)
    conv_w = np.asarray(conv_w, np.float32)
    conv_b = np.asarray(conv_b, np.float32)
    W_x = np.asarray(W_x, np.float32)
    W_dt = np.asarray(W_dt, np.float32)
    b_dt = np.asarray(b_dt, np.float32)
    A_log = np.asarray(A_log, np.float32)
    D = np.asarray(D, np.float32)
    W_out = np.asarray(W_out, np.float32)

    xt = np.ascontiguousarray(x.transpose(0, 2, 1)).reshape(B, KBLK, 128, L).astype(ml_dtypes.bfloat16)
    A = np.exp(A_log)  # positive |A|; md = -dt on device

    in_maps = []
    for c in range(NCORES):
        lo = c * DIL
        sl = slice(lo, lo + DIL)
        cw = conv_w[sl]  # [DIL, DC]
        diagk = np.zeros((DC, DIL, 128), np.float32)
        for k in range(DC):
            for blk in range(NBLK):
                diagk[k, blk * 128:(blk + 1) * 128, :] = np.diag(
                    cw[blk * 128:(blk + 1) * 128, k])
        in_maps.append({
            "x_t": xt,
            "win": np.ascontiguousarray(
                np.concatenate([W_in[:, sl], W_in[:, DI + lo:DI + lo + DIL]],
                               axis=1)).astype(ml_dtypes.bfloat16),
            "wout": np.ascontiguousarray(W_out[sl]).astype(ml_dtypes.bfloat16),
            "wx": np.ascontiguousarray(
                np.concatenate([W_x[sl, :DS], -W_x[sl, DS:]], axis=1)
            ).astype(ml_dtypes.bfloat16),
            "wdt": np.ascontiguousarray(W_dt[:, sl]).astype(ml_dtypes.bfloat16),
            "a": np.ascontiguousarray(A[sl]),
            "convb": np.ascontiguousarray(conv_b[sl, None]),
            "convw": np.ascontiguousarray(cw),
            "bdt": np.ascontiguousarray(-b_dt[sl, None]),
            "identb": np.eye(128, dtype=ml_dtypes.bfloat16),
            "diagd": np.stack([np.diag(D[lo + k * 128:lo + (k + 1) * 128])
                               for k in range(NBLK)]).reshape(DIL, 128)
                       .astype(ml_dtypes.bfloat16),
            "diagk": diagk.astype(ml_dtypes.bfloat16),
        })
    return in_maps


def kernel(**inputs):
    nc = _get_nc()
    in_maps = make_in_maps(**inputs)
    res = run_bass_kernel_spmd(nc, in_maps, list(range(NCORES)))
    out = np.zeros((B, L, DM), np.float32)
    for c in range(NCORES):
        out += res.results[c]["out_p"]
    return out


# revision 35
# speedup vs baseline: 1.0415x; 1.0415x over previous
"""Mamba block (MockMambaBlock) on 8 Trainium2 NeuronCores.

Sharding: tensor-parallel over d_inner (8 x 256 channels), both batches on
every core. The x_proj/dt_proj contraction over d_inner is completed with an
on-device AllReduce of the small (B, 32, L) partial; out_proj row-partials
are summed on the host (the gather step).

v4: conv on the PE (per-tap diagonal matmuls); AllReduce split into L/2
halves so the scan chain starts ~40us earlier; full-width (N=2048) identity
and diagd matmuls into a single 4-bank PSUM accumulator; first/last blocks
scanned in chained halves so gating + out_proj pipeline into head/tail;
phase A of batch 1 emitted interleaved with the batch-0 scan chain.
"""

import sys

sys.path.insert(0, "/opt/trn_rl_repo")

import numpy as np
import ml_dtypes

import concourse.bass as bass
import concourse.bacc as bacc
import concourse.mybir as mybir
import concourse.tile as tile
from concourse.bass_utils import run_bass_kernel_spmd

F32 = mybir.dt.float32
BF16 = mybir.dt.bfloat16
AF = mybir.ActivationFunctionType
OP = mybir.AluOpType

B, L, DM, DI, DS, DC = 2, 2048, 1024, 2048, 16, 4
NCORES = 8
DIL = DI // NCORES          # 256 channels per core
NBLK = DIL // 128           # 2 partition blocks of channels
KBLK = DM // 128            # 8 contraction blocks for in_proj
LTA = 512                   # phase A token chunk
LH = L // 2                 # AllReduce / scan half


def build_nc():
    nc = bacc.Bacc()

    x_t = nc.dram_tensor("x_t", [B, KBLK, 128, L], BF16, kind="ExternalInput")
    win_d = nc.dram_tensor("win", [DM, 2 * DIL], BF16, kind="ExternalInput")
    wout_d = nc.dram_tensor("wout", [DIL, DM], BF16, kind="ExternalInput")
    wx_d = nc.dram_tensor("wx", [DIL, 2 * DS], BF16, kind="ExternalInput")
    wdt_d = nc.dram_tensor("wdt", [DS, DIL], BF16, kind="ExternalInput")
    a_d = nc.dram_tensor("a", [DIL, DS], F32, kind="ExternalInput")
    convb_d = nc.dram_tensor("convb", [DIL, 1], F32, kind="ExternalInput")
    convw_d = nc.dram_tensor("convw", [DIL, DC], F32, kind="ExternalInput")
    bdt_d = nc.dram_tensor("bdt", [DIL, 1], F32, kind="ExternalInput")
    identb_d = nc.dram_tensor("identb", [128, 128], BF16, kind="ExternalInput")
    diagd_d = nc.dram_tensor("diagd", [DIL, 128], BF16, kind="ExternalInput")
    diagk_d = nc.dram_tensor("diagk", [DC, DIL, 128], BF16, kind="ExternalInput")
    out_d = nc.dram_tensor("out_p", [B, L, DM], F32, kind="ExternalOutput")

    ncha = L // LTA

    with tile.TileContext(nc) as tc:
        with (
            tc.tile_pool(name="weights", bufs=1) as wp,
            tc.tile_pool(name="resident", bufs=1) as rp,
            tc.tile_pool(name="dram", bufs=1, space="DRAM") as dp,
        ):
            # ---- weights to SBUF ----
            win_sb = wp.tile([128, KBLK, 2 * DIL], BF16)
            nc.sync.dma_start(win_sb[:], win_d[:].rearrange("(k p) m -> p k m", p=128))
            wout_sb = wp.tile([128, NBLK, DM], BF16)
            nc.sync.dma_start(wout_sb[:], wout_d[:].rearrange("(k p) m -> p k m", p=128))
            wx_sb = wp.tile([128, NBLK, 2 * DS], BF16)
            nc.sync.dma_start(wx_sb[:], wx_d[:].rearrange("(k p) m -> p k m", p=128))
            wdt_sb = wp.tile([DS, DIL], BF16)
            nc.sync.dma_start(wdt_sb[:], wdt_d[:])
            a_sb = wp.tile([128, NBLK, DS], F32)
            nc.sync.dma_start(a_sb[:], a_d[:].rearrange("(k p) m -> p k m", p=128))
            convb_sb = wp.tile([128, NBLK, 1], F32)
            nc.sync.dma_start(convb_sb[:], convb_d[:].rearrange("(k p) m -> p k m", p=128))
            convw_sb = wp.tile([128, NBLK, DC], F32)
            nc.sync.dma_start(convw_sb[:], convw_d[:].rearrange("(k p) m -> p k m", p=128))
            bdt_sb = wp.tile([128, NBLK, 1], F32)
            nc.sync.dma_start(bdt_sb[:], bdt_d[:].rearrange("(k p) m -> p k m", p=128))
            identb_sb = wp.tile([128, 128], BF16)
            nc.sync.dma_start(identb_sb[:], identb_d[:])
            diagd_sb = wp.tile([128, NBLK, 128], BF16)
            nc.sync.dma_start(diagd_sb[:], diagd_d[:].rearrange("(k p) m -> p k m", p=128))
            diagk_sb = wp.tile([128, DC, NBLK, 128], BF16)
            nc.sync.dma_start(
                diagk_sb[:],
                diagk_d[:].rearrange("c (k p) m -> p c k m", p=128))

            # ---- resident activations (both batches) ----
            xcv = [[rp.tile([128, L], BF16, name=f"xcv{b_}{k}", tag=f"xcv{b_}{k}")
                    for k in range(NBLK)] for b_ in range(B)]
            zac = [[rp.tile([128, L], BF16, name=f"zac{b_}{k}", tag=f"zac{b_}{k}")
                    for k in range(NBLK)] for b_ in range(B)]
            md = [[rp.tile([128, L], BF16, name=f"md{b_}{k}", tag=f"md{b_}{k}")
                   for k in range(NBLK)] for b_ in range(B)]
            dtin_sb = [rp.tile([DS, L], BF16, name=f"dtin{b_}", tag=f"dtin{b_}")
                       for b_ in range(B)]
            xp = [[rp.tile([128, L + DC - 1], BF16, name=f"xp{b_}{k}",
                           tag=f"xp{b_}{k}") for k in range(NBLK)]
                  for b_ in range(B)]
            yin = [[rp.tile([128, L], BF16, name=f"yin{b_}{k}", tag=f"yin{b_}{k}")
                    for k in range(NBLK)] for b_ in range(B)]

            # collective buffers, one per (batch, token-range). Batch 0 uses
            # finer leading ranges so its scan chain can start early.
            RNG = {0: [(0, 512), (512, 1024), (1024, 2048)],
                   1: [(0, 1024), (1024, 2048)]}
            cc_in = {b_: [dp.tile([2 * DS, r1 - r0], BF16,
                                  name=f"cc_in{b_}_{r0}")
                          for (r0, r1) in RNG[b_]] for b_ in range(B)}
            cc_out = {b_: [dp.tile([2 * DS, r1 - r0], BF16,
                                   addr_space="Shared", name=f"cc_out{b_}_{r0}")
                           for (r0, r1) in RNG[b_]] for b_ in range(B)}

            # PSUM budget (8 banks): ps_in(2) + cps(1) + ps_xs(1) + y_ps(4).
            # ps_dt and ps_o reuse the ps_in tag.
            pools_cm = (
                tc.tile_pool(name="pa", bufs=2),
                tc.tile_pool(name="pa_ps", bufs=2, space="PSUM"),
                tc.tile_pool(name="pb", bufs=2),
                tc.tile_pool(name="pb_ps", bufs=1, space="PSUM"),
            )
            pa = pools_cm[0].__enter__()
            paps = pools_cm[1].__enter__()
            pb = pools_cm[2].__enter__()
            pbps = pools_cm[3].__enter__()

            def phase_a_chunk(b_, ch):
                t0 = ch * LTA
                xs_all = pa.tile([128, KBLK, LTA], BF16, tag="xs_all", bufs=3)
                nc.sync.dma_start(
                    xs_all[:],
                    x_t[b_].transpose([1, 0, 2])[:, :, t0:t0 + LTA])
                for m in range(2 * NBLK):
                    ps = paps.tile([128, LTA], F32, tag="ps_in", bufs=2)
                    for kb in range(KBLK):
                        nc.tensor.matmul(
                            ps[:],
                            win_sb[:, kb, m * 128:(m + 1) * 128],
                            xs_all[:, kb, :],
                            start=(kb == 0), stop=(kb == KBLK - 1))
                    if m < NBLK:  # x branch: conv (PE diag matmuls, or DVE
                        # scalar_tensor_tensor for batch 0 where DVE idles)
                        blk = m
                        if ch == 0:
                            nc.vector.memset(xp[b_][blk][:, 0:DC - 1], 0.0)
                        nc.scalar.copy(
                            xp[b_][blk][:, DC - 1 + t0:DC - 1 + t0 + LTA], ps[:])
                        if b_ == 0:
                            cacc = pa.tile([128, LTA], F32, tag="cacc", bufs=2)
                            nc.vector.tensor_scalar_mul(
                                cacc[:], xp[b_][blk][:, t0:t0 + LTA],
                                convw_sb[:, blk, 0:1])
                            for k in range(1, DC):
                                nc.vector.scalar_tensor_tensor(
                                    cacc[:], xp[b_][blk][:, t0 + k:t0 + k + LTA],
                                    convw_sb[:, blk, k:k + 1], cacc[:],
                                    OP.mult, OP.add)
                            nc.scalar.activation(
                                xcv[b_][blk][:, t0:t0 + LTA], cacc[:],
                                AF.Silu, bias=convb_sb[:, blk, :])
                        else:
                            cps = paps.tile([128, LTA], F32, tag="cps", bufs=1)
                            for k in range(DC):
                                nc.tensor.matmul(
                                    cps[:],
                                    diagk_sb[:, k, blk, :],
                                    xp[b_][blk][:, t0 + k:t0 + k + LTA],
                                    start=(k == 0), stop=(k == DC - 1))
                            nc.scalar.activation(
                                xcv[b_][blk][:, t0:t0 + LTA], cps[:],
                                AF.Silu, bias=convb_sb[:, blk, :])
                    else:  # z branch: silu
                        blk = m - NBLK
                        nc.scalar.activation(
                            zac[b_][blk][:, t0:t0 + LTA], ps[:], AF.Silu)
                # x_proj partial for this chunk (high priority for batch 0:
                # it feeds the AllReduce on the critical path)
                import contextlib
                hp = tc.high_priority() if b_ == 0 else contextlib.nullcontext()
                with hp:
                    ps_xs = paps.tile([2 * DS, LTA], F32, tag="ps_xs", bufs=1)
                    for kb in range(NBLK):
                        nc.tensor.matmul(
                            ps_xs[:],
                            wx_sb[:, kb, :],
                            xcv[b_][kb][:, t0:t0 + LTA],
                            start=(kb == 0), stop=(kb == NBLK - 1))
                    xs_sb = pa.tile([2 * DS, LTA], BF16, tag="xs_sb", bufs=2)
                    nc.scalar.copy(xs_sb[:], ps_xs[:])
                    # scalar HWDGE queue: clear of the big xs_all loads
                    for ri, (r0, r1) in enumerate(RNG[b_]):
                        if r0 <= t0 < r1:
                            nc.scalar.dma_start(
                                cc_in[b_][ri][:, t0 - r0:t0 - r0 + LTA],
                                xs_sb[:])

            def all_reduce(b_, ri):
                r0, r1 = RNG[b_][ri]
                nc.gpsimd.collective_compute(
                    "AllReduce", OP.add,
                    ins=[cc_in[b_][ri].opt()], outs=[cc_out[b_][ri].opt()],
                    replica_groups=[list(range(NCORES))])
                nc.scalar.dma_start(dtin_sb[b_][:, r0:r1],
                                    cc_out[b_][ri][0:DS, :])

            def dt_phase(b_, ri):
                # md = -softplus(dt_raw + b_dt) = ln(sigmoid(-(dt_raw + b_dt)))
                LTD = 512
                r0, r1 = RNG[b_][ri]
                for blk in range(NBLK):
                    for ch in range((r1 - r0) // LTD):
                        t0 = r0 + ch * LTD
                        ps_dt = paps.tile([128, LTD], F32, tag="ps_in", bufs=2)
                        nc.tensor.matmul(
                            ps_dt[:], wdt_sb[:, blk * 128:(blk + 1) * 128],
                            dtin_sb[b_][:, t0:t0 + LTD],
                            start=True, stop=True)
                        nc.scalar.activation(
                            md[b_][blk][:, t0:t0 + LTD], ps_dt[:],
                            AF.Sigmoid, bias=bdt_sb[:, blk, :], scale=-1.0)
                for blk in range(NBLK):
                    nc.scalar.activation(md[b_][blk][:, r0:r1],
                                         md[b_][blk][:, r0:r1], AF.Ln)

            def make_dtx(b_, blk, on_dve=False, t0=0, t1=L, dtx=None):
                if dtx is None:
                    dtx = pb.tile([128, L], BF16, tag="dtx", bufs=2,
                                  name=f"dtx{b_}{blk}")
                nc.vector.tensor_mul(dtx[:, t0:t1], md[b_][blk][:, t0:t1],
                                     xcv[b_][blk][:, t0:t1])
                return dtx

            def make_yps(b_, blk):
                y_ps = pbps.tile([128, L], F32, tag="y_ps", bufs=1,
                                 name=f"yps{b_}{blk}")
                for pt in range(L // 512):
                    nc.tensor.matmul(y_ps[:, pt * 512:(pt + 1) * 512],
                                     diagd_sb[:, blk, :],
                                     xcv[b_][blk][:, pt * 512:(pt + 1) * 512],
                                     start=True, stop=False)
                return y_ps

            # states whose y-accumulation runs as DVE tree-adds instead of PE
            # identity matmuls (PE/DVE load balance)
            DVE_SUM = set(range(DS - 4, DS))

            def phase_b_n(b_, blk, n, dtx, y_ps, t0=0, t1=L, carry=None,
                          save_carry=None, hstate=None):
                tl = t1 - t0
                bb = pb.tile([128, tl], BF16, tag="bbn", bufs=3,
                             name=f"bb{b_}{blk}{n}{t0}")
                done = 0
                for ri, (r0, r1) in enumerate(RNG[b_]):
                    o0, o1 = max(t0, r0), min(t1, r1)
                    if o0 < o1:
                        # pool SWDGE queue: avoids contention with xs_all loads
                        nc.gpsimd.dma_start(
                            bb[:, o0 - t0:o1 - t0],
                            cc_out[b_][ri][DS + n:DS + n + 1, o0 - r0:o1 - r0]
                            .broadcast_to([128, o1 - o0]))
                        done += o1 - o0
                assert done == tl
                da = pb.tile([128, tl], F32, tag="dan", bufs=2,
                             name=f"da{b_}{blk}{n}{t0}")
                nc.scalar.activation(da[:], md[b_][blk][:, t0:t1], AF.Exp,
                                     scale=a_sb[:, blk, n:n + 1])
                u = pb.tile([128, tl], BF16, tag="un", bufs=3,
                            name=f"u{b_}{blk}{n}{t0}")
                nc.vector.tensor_mul(u[:], dtx[:, t0:t1], bb[:])
                h = pb.tile([128, tl], BF16, tag="hn", bufs=2,
                            name=f"h{b_}{blk}{n}{t0}")
                init = 0.0 if carry is None else carry[:, n:n + 1]
                nc.vector.tensor_tensor_scan(h[:], da[:], u[:],
                                             init, OP.mult, OP.add)
                if save_carry is not None:
                    nc.vector.tensor_copy(save_carry[:, n:n + 1], h[:, tl - 1:tl])
                if n in DVE_SUM:
                    # fold into the DVE partial sum; last state emits the
                    # shadow identity matmul with the stop flag
                    first = min(DVE_SUM)
                    if n == first:
                        hstate["h0"] = h
                    elif n == first + 1:
                        S = pb.tile([128, tl], BF16, tag="hsum", bufs=2,
                                    name=f"hs{b_}{blk}{t0}")
                        nc.vector.tensor_add(S[:], hstate.pop("h0")[:], h[:])
                        hstate["S"] = S
                    else:
                        S = hstate["S"]
                        nc.vector.tensor_add(S[:], S[:], h[:])
                    if n == DS - 1:
                        S = hstate.pop("S")
                        for pt in range(tl // 512):
                            nc.tensor.matmul(
                                y_ps[:, t0 + pt * 512:t0 + (pt + 1) * 512],
                                identb_sb[:], S[:, pt * 512:(pt + 1) * 512],
                                start=False, stop=True)
                else:
                    for pt in range(tl // 512):
                        nc.tensor.matmul(
                            y_ps[:, t0 + pt * 512:t0 + (pt + 1) * 512],
                            identb_sb[:], h[:, pt * 512:(pt + 1) * 512],
                            start=False, stop=False)

            def phase_b_gate(b_, blk, y_ps, t0=0, t1=L):
                nc.vector.tensor_mul(
                    yin[b_][blk][:, t0:t1], y_ps[:, t0:t1],
                    zac[b_][blk][:, t0:t1])

            def out_proj(b_, mts):
                for mt in mts:
                    for dmh in range(2):
                        ps_o = paps.tile([128, 512], F32, tag="ps_in", bufs=2)
                        for blk in range(NBLK):
                            nc.tensor.matmul(
                                ps_o[:],
                                yin[b_][blk][:, mt * 128:(mt + 1) * 128],
                                wout_sb[:, blk, dmh * 512:(dmh + 1) * 512],
                                start=(blk == 0), stop=(blk == NBLK - 1))
                        osb = pb.tile([128, 512], F32, tag="osb", bufs=2)
                        nc.scalar.copy(osb[:], ps_o[:])
                        nc.sync.dma_start(
                            out_d[b_, mt * 128:(mt + 1) * 128,
                                  dmh * 512:(dmh + 1) * 512],
                            osb[:])

            # ---------------- emission schedule ----------------
            phase_a_chunk(0, 0)
            with tc.high_priority():
                all_reduce(0, 0)
            phase_a_chunk(0, 1)
            with tc.high_priority(offset=500):
                all_reduce(0, 1)
            phase_a_chunk(0, 2)
            phase_a_chunk(0, 3)
            with tc.high_priority(offset=500):
                all_reduce(0, 2)
            with tc.high_priority():
                dt_phase(0, 0)
                dtx00 = make_dtx(0, 0, on_dve=True, t0=0, t1=512)
            y00 = make_yps(0, 0)
            carry00 = pb.tile([128, DS], BF16, tag="carry", bufs=2,
                              name="carry00")
            # B(0,0) segment 1 as early as possible
            hs = {}
            for n in range(DS):
                phase_b_n(0, 0, n, dtx00, y00, 0, 512, save_carry=carry00,
                          hstate=hs)
            with tc.high_priority(offset=800):
                dt_phase(0, 1)
                make_dtx(0, 0, on_dve=True, t0=512, t1=1024, dtx=dtx00)
            phase_b_gate(0, 0, y00, 0, 512)
            hs = {}
            for n in range(DS):
                phase_b_n(0, 0, n, dtx00, y00, 512, 1024, carry=carry00,
                          save_carry=carry00, hstate=hs)
            with tc.high_priority(offset=800):
                dt_phase(0, 2)
                make_dtx(0, 0, on_dve=True, t0=1024, t1=L, dtx=dtx00)
            phase_b_gate(0, 0, y00, 512, 1024)
            # segment 3 interleaved with phase A of batch 1
            hs = {}
            nxt = 0
            for ch in range(ncha):
                phase_a_chunk(1, ch)
                if ch == 1:
                    all_reduce(1, 0)
                if ch == 3:
                    all_reduce(1, 1)
                for n in range(nxt, nxt + 4):
                    phase_b_n(0, 0, n, dtx00, y00, 1024, L, carry=carry00,
                              hstate=hs)
                nxt += 4
            phase_b_gate(0, 0, y00, 1024, L)
            # B(0,1) full-length
            dtx01 = make_dtx(0, 1)
            y01 = make_yps(0, 1)
            hs = {}
            phase_b_n(0, 1, 0, dtx01, y01, hstate=hs)
            phase_b_n(0, 1, 1, dtx01, y01, hstate=hs)
            dt_phase(1, 0)
            dt_phase(1, 1)
            for n in range(2, DS):
                phase_b_n(0, 1, n, dtx01, y01, hstate=hs)
            phase_b_gate(0, 1, y01)
            # B(1,0) with out_proj(0) interleaved
            dtx10 = make_dtx(1, 0)
            y10 = make_yps(1, 0)
            hs = {}
            for n in range(DS):
                phase_b_n(1, 0, n, dtx10, y10, hstate=hs)
                if n < 8:
                    out_proj(0, mts=[2 * n, 2 * n + 1])
            phase_b_gate(1, 0, y10)
            # B(1,1) in chained halves; out_proj(1) rides the second half
            dtx11 = make_dtx(1, 1)
            y11 = make_yps(1, 1)
            carry11 = pb.tile([128, DS], BF16, tag="carry", bufs=2,
                              name="carry11")
            hs = {}
            for n in range(DS):
                phase_b_n(1, 1, n, dtx11, y11, 0, LH, save_carry=carry11,
                          hstate=hs)
            phase_b_gate(1, 1, y11, 0, LH)
            hs = {}
            for n in range(DS):
                phase_b_n(1, 1, n, dtx11, y11, LH, L, carry=carry11,
                          hstate=hs)
                if n < 8:
                    out_proj(1, mts=[2 * n, 2 * n + 1])
            phase_b_gate(1, 1, y11, LH, L)
            out_proj(1, mts=range(8, 16))

            for cm in reversed(pools_cm):
                cm.__exit__(None, None, None)

    nc.compile()
    return nc


_NC_CACHE = {}


def _get_nc():
    if "nc" not in _NC_CACHE:
        _NC_CACHE["nc"] = build_nc()
    return _NC_CACHE["nc"]


def make_in_maps(x, W_in, conv_w, conv_b, W_x, W_dt, b_dt, A_log, D, W_out):
    x = np.asarray(x, np.float32)
    W_in = np.asarray(W_in, np.float32)
    conv_w = np.asarray(conv_w, np.float32)
    conv_b = np.asarray(conv_b, np.float32)
    W_x = np.asarray(W_x, np.float32)
    W_dt = np.asarray(W_dt, np.float32)
    b_dt = np.asarray(b_dt, np.float32)
    A_log = np.asarray(A_log, np.float32)
    D = np.asarray(D, np.float32)
    W_out = np.asarray(W_out, np.float32)

    xt = np.ascontiguousarray(x.transpose(0, 2, 1)).reshape(B, KBLK, 128, L).astype(ml_dtypes.bfloat16)
    A = np.exp(A_log)  # positive |A|; md = -dt on device

    in_maps = []
    for c in range(NCORES):
        lo = c * DIL
        sl = slice(lo, lo + DIL)
        cw = conv_w[sl]  # [DIL, DC]
        diagk = np.zeros((DC, DIL, 128), np.float32)
        for k in range(DC):
            for blk in range(NBLK):
                diagk[k, blk * 128:(blk + 1) * 128, :] = np.diag(
                    cw[blk * 128:(blk + 1) * 128, k])
        in_maps.append({
            "x_t": xt,
            "win": np.ascontiguousarray(
                np.concatenate([W_in[:, sl], W_in[:, DI + lo:DI + lo + DIL]],
                               axis=1)).astype(ml_dtypes.bfloat16),
            "wout": np.ascontiguousarray(W_out[sl]).astype(ml_dtypes.bfloat16),
            "wx": np.ascontiguousarray(
                np.concatenate([W_x[sl, :DS], -W_x[sl, DS:]], axis=1)
            ).astype(ml_dtypes.bfloat16),
            "wdt": np.ascontiguousarray(W_dt[:, sl]).astype(ml_dtypes.bfloat16),
            "a": np.ascontiguousarray(A[sl]),
            "convb": np.ascontiguousarray(conv_b[sl, None]),
            "convw": np.ascontiguousarray(cw),
            "bdt": np.ascontiguousarray(-b_dt[sl, None]),
            "identb": np.eye(128, dtype=ml_dtypes.bfloat16),
            "diagd": np.stack([np.diag(D[lo + k * 128:lo + (k + 1) * 128])
                               for k in range(NBLK)]).reshape(DIL, 128)
                       .astype(ml_dtypes.bfloat16),
            "diagk": diagk.astype(ml_dtypes.bfloat16),
        })
    return in_maps


def kernel(**inputs):
    nc = _get_nc()
    in_maps = make_in_maps(**inputs)
    res = run_bass_kernel_spmd(nc, in_maps, list(range(NCORES)))
    out = np.zeros((B, L, DM), np.float32)
    for c in range(NCORES):
        out += res.results[c]["out_p"]
    return out
